# revision 26
# baseline (speedup 1.0000x reference)
"""GCN binding-affinity model on 8 Trainium2 NeuronCores.

Self-contained: builds a Bass/Tile SPMD program, shards the graph across 8
cores (nodes dst-partitioned; per-core edges packed into fixed 128-edge tiles
per 128-node window, split by permuted-row half for int16 dma_gather indices),
runs via bass_utils.run_bass_kernel_spmd, and returns the full [256, 1] output.

Math (equivalent to the reference):
  per layer: agg = dinv * (A @ (dinv * t) + dinv * t) with t the passed
  features, where dinv = rsqrt(indegree + 1).  Self-loop terms are added
  on-chip from the previous layer's per-window tiles (no self edges in the
  gather tables).
  L1 passes t = x (scalar; table stores y = dinv*x in bf16), then
  h1 = relu(dinv*(S1 + y) * W1 + b1) via K=1 outer product.
  L2 passes t = h1 (64-d), then h2 = relu(dinv*(S2 + g2) @ W2 + b2).
  L3 passes t = h2 @ W3 (64-d), then h3 = relu(dinv*(S3 + g3) + b3).
  pool: one-hot matmul by graph id + AllReduce + mean + 2-layer MLP.

Data layout: gather tables are [PAD_N, 128] bf16 rows (256 B pitch; features
in cols 0..64, junk above) indexed by *permuted* rows so that chunked
AllGathers land contiguously: chunk 0 = windows [0, CW) of every core, chunk 1
= the rest.  The first table (y) is built locally on every core from the
replicated x input, so only two AllGathers remain and each is split in two
chunks overlapping the producing layer's compute.
"""

import os
import sys
from contextlib import ExitStack

import numpy as np
import ml_dtypes

for _p in ("/opt/trn_rl_repo",):
    if _p not in sys.path and os.path.isdir(_p):
        sys.path.insert(0, _p)

import concourse.bass as bass
import concourse.mybir as mybir
import concourse.tile as tile
from concourse import bacc
from concourse import bass_utils
from concourse.masks import make_identity
from concourse.tile_rust import add_dep_helper

F32 = mybir.dt.float32
BF16 = mybir.dt.bfloat16
I16 = mybir.dt.int16
AF = mybir.ActivationFunctionType
OP = mybir.AluOpType
BF = ml_dtypes.bfloat16
GSZ = 4   # windows per gather group
CW = 36   # windows in AllGather chunk 0 (must be a multiple of GSZ)


class Cfg:
    def __init__(self, n_nodes=50000, n_edges=600000, n_graphs=256,
                 n_cores=8, nw=49, nt_a=9, nt_b=5, half=32768, gsz=GSZ,
                 cw=CW):
        self.N = n_nodes
        self.E = n_edges
        self.G = n_graphs
        self.C = n_cores
        self.NW = nw               # 128-node windows per core
        self.NT_A = nt_a           # tiles/window for permrow < half
        self.NT_B = nt_b           # tiles/window for permrow >= half
        self.NT = nt_a + nt_b
        self.HALF = half
        self.NWS = nw * 128
        self.PAD_N = self.C * self.NWS
        self.CW = cw
        self.CH0 = self.C * cw * 128      # rows in AllGather chunk 0
        assert self.PAD_N >= n_nodes
        assert self.PAD_N - half <= 32768 and half <= 32768
        self.GP = 256
        assert n_graphs <= self.GP
        self.GSZ = gsz
        self.groups = [(s, min(gsz, nw - s)) for s in range(0, nw, gsz)]

    def key(self):
        return (self.N, self.E, self.G, self.C, self.NW, self.NT_A,
                self.NT_B, self.HALF, self.GSZ, self.CW)


def chunk_of(cfg, wl, t, gn):
    """gb chunk index for window-in-group wl, window-tile t, group size gn."""
    if t < cfg.NT_A:
        return wl * cfg.NT_A + t
    return gn * cfg.NT_A + wl * cfg.NT_B + (t - cfg.NT_A)


def permrow(cfg, n):
    """global padded node id -> permuted gather-table row (chunked AG order)."""
    n = np.asarray(n, dtype=np.int64)
    c = n // cfg.NWS
    r = n - c * cfg.NWS
    w = r >> 7
    p = r & 127
    row0 = c * (cfg.CW * 128) + (w << 7) + p
    row1 = (cfg.CH0 + c * ((cfg.NW - cfg.CW) * 128)
            + ((w - cfg.CW) << 7) + p)
    return np.where(w < cfg.CW, row0, row1)


# ----------------------------------------------------------------------------
# Host-side sharding / packing
# ----------------------------------------------------------------------------

def compute_caps(n, edge_index, cfg_proto):
    """Data-driven NT_A/NT_B (max tiles needed per window, no self-loops)."""
    src = np.asarray(edge_index[0], dtype=np.int64)
    dst = np.asarray(edge_index[1], dtype=np.int64)
    pr = permrow(cfg_proto, src)
    win = dst >> 7  # global 128-node window
    n_win = cfg_proto.C * cfg_proto.NW
    isa = pr < cfg_proto.HALF
    cnt_a = np.bincount(win[isa], minlength=n_win)
    cnt_b = np.bincount(win[~isa], minlength=n_win)
    nt_a = max(int(np.ceil(cnt_a.max() / 128)), 1)
    nt_b = max(int(np.ceil(cnt_b.max() / 128)), 1)
    return nt_a, nt_b


def wrap16(vals):
    """sequence position i -> [i % 16, i // 16], int16."""
    return np.ascontiguousarray(vals.reshape(-1, 16).T.astype(np.int16))


def rep8(block):
    """replicate a [16, X] int16 block to [128, X] (8 Q7 core groups)."""
    return np.tile(block, (8, 1))


def prep_inputs(cfg, x, W1, b1, W2, b2, W3, b3, lin1_w, lin1_b, lin2_w,
                lin2_b, edge_index, batch):
    N, C, NW, NWS = cfg.N, cfg.C, cfg.NW, cfg.NWS
    NT_A, NT_B, NT, HALF = cfg.NT_A, cfg.NT_B, cfg.NT, cfg.HALF

    src = np.asarray(edge_index[0], dtype=np.int64)
    dst = np.asarray(edge_index[1], dtype=np.int64)
    batch = np.asarray(batch, dtype=np.int64)
    x = np.asarray(x, dtype=np.float32).reshape(-1)

    deg = np.bincount(dst, minlength=N).astype(np.float32)
    x_ext = np.zeros(cfg.PAD_N, np.float32); x_ext[:N] = x
    dinv_ext = np.ones(cfg.PAD_N, np.float32)
    dinv_ext[:N] = 1.0 / np.sqrt(deg + 1.0)
    batch_ext = np.full(cfg.PAD_N, -1.0, np.float32)
    batch_ext[:N] = batch.astype(np.float32)

    iota = np.broadcast_to(np.arange(cfg.GP, dtype=np.float32),
                           (128, cfg.GP)).astype(BF).copy()
    cnts = np.bincount(batch, minlength=cfg.GP).astype(np.float32)
    cnts2 = np.ascontiguousarray(cnts.reshape(2, 128).T)  # [128, half]

    # permuted-row inverse: node id for each table row
    rows_of = permrow(cfg, np.arange(cfg.PAD_N, dtype=np.int64))
    nor = np.empty(cfg.PAD_N, np.int64)
    nor[rows_of] = np.arange(cfg.PAD_N, dtype=np.int64)
    xt_full = np.ascontiguousarray(
        x_ext[nor].reshape(cfg.PAD_N // 128, 128).T)
    dinv_full = np.ascontiguousarray(
        dinv_ext[nor].reshape(cfg.PAD_N // 128, 128).T)

    # assign edges to cores by dst (no self-loops in tables)
    a_src, a_dst = src, dst
    core_of = a_dst // NWS
    order0 = np.argsort(core_of, kind="stable")
    a_src, a_dst, core_sorted = a_src[order0], a_dst[order0], core_of[order0]
    pr_src = permrow(cfg, a_src)

    # per-group column offsets in the idx arrays
    colsA_of = [gn * NT_A * 128 // 16 for _, gn in cfg.groups]
    colsB_of = [gn * NT_B * 128 // 16 for _, gn in cfg.groups]
    offA = np.concatenate([[0], np.cumsum(colsA_of)]).astype(int)
    offB = np.concatenate([[0], np.cumsum(colsB_of)]).astype(int)
    TOT_A, TOT_B = int(offA[-1]), int(offB[-1])

    in_maps = []
    for c in range(C):
        base = c * NWS
        lo = np.searchsorted(core_sorted, c, side="left")
        hi = np.searchsorted(core_sorted, c, side="right")
        c_src, c_dst, c_pr = a_src[lo:hi], a_dst[lo:hi], pr_src[lo:hi]
        w_of = (c_dst - base) >> 7
        is_a = c_pr < HALF
        # order edges by (window, half), then rank within each bucket
        key = w_of * 2 + (~is_a)
        order = np.argsort(key, kind="stable")
        c_pr, c_dst, w_of, is_a = (c_pr[order], c_dst[order], w_of[order],
                                   is_a[order])
        key = key[order]
        kstart = np.searchsorted(key, np.arange(2 * NW), side="left")
        pos = np.arange(len(key)) - kstart[key]
        cnt = np.searchsorted(key, np.arange(2 * NW), side="right") - kstart
        if cnt[0::2].max(initial=0) > NT_A * 128 or \
           cnt[1::2].max(initial=0) > NT_B * 128:
            raise ValueError("window half overflow; increase caps")

        # per-window slot arrays: slots [0, NT_A*128) half A, rest half B
        slot_idx = np.zeros((NW, NT * 128), np.int64)      # biased table row
        dstrel = np.full((NW, NT * 128), -1.0, np.float32)
        wslot = np.where(is_a, pos, NT_A * 128 + pos)
        slot_idx[w_of, wslot] = np.where(is_a, c_pr, c_pr - HALF)
        dstrel[w_of, wslot] = (c_dst - base - (w_of << 7)).astype(np.float32)

        ixa = np.zeros((128, TOT_A), np.int16)
        ixb = np.zeros((128, TOT_B), np.int16)
        for gi, (ws, gn) in enumerate(cfg.groups):
            seq_a = slot_idx[ws:ws + gn, :NT_A * 128].reshape(-1)
            seq_b = slot_idx[ws:ws + gn, NT_A * 128:].reshape(-1)
            ixa[:, offA[gi]:offA[gi + 1]] = rep8(wrap16(seq_a))
            ixb[:, offB[gi]:offB[gi + 1]] = rep8(wrap16(seq_b))

        # dstrel as SBUF layout [128, NW*NT]: col w*NT+t, partition p
        drel = np.ascontiguousarray(
            dstrel.reshape(NW * NT, 128).T).astype(BF)

        sl = slice(base, base + NWS)
        nd_batch = np.ascontiguousarray(
            batch_ext[sl].reshape(NW, 128).T).astype(BF)
        dinvT = np.broadcast_to(dinv_ext[sl][None, :], (64, NWS)).astype(BF).copy()
        x_row = np.ascontiguousarray(x_ext[sl].reshape(1, NWS)).astype(BF)
        dinv_row = np.ascontiguousarray(
            dinv_ext[sl].reshape(1, NWS)).astype(BF)

        in_maps.append({
            "ixa": ixa, "ixb": ixb,
            "ei_dstrel": drel,
            "xt_full": xt_full,
            "dinv_full": dinv_full,
            "x_row": x_row,
            "dinv_row": dinv_row,
            "dinvT": dinvT,
            "nd_batch": nd_batch,
            "iota": iota,
            "cnts": cnts2,
            "w1b": np.asarray(W1, np.float32).reshape(1, 64).astype(BF),
            "w2b": np.asarray(W2, np.float32).reshape(64, 128).astype(BF),
            "w3b": np.asarray(W3, np.float32).reshape(128, 64).astype(BF),
            "b1": np.asarray(b1, np.float32).reshape(64, 1),
            "b2": np.asarray(b2, np.float32).reshape(128, 1),
            "b3": np.asarray(b3, np.float32).reshape(64, 1),
            "l1w": np.asarray(lin1_w, np.float32).reshape(64, 32),
            "l1b": np.asarray(lin1_b, np.float32).reshape(32, 1),
            "l2w": np.asarray(lin2_w, np.float32).reshape(32, 1),
            "l2b": np.full((128, 1),
                           np.float32(np.asarray(lin2_b).reshape(())),
                           np.float32),
        })
    return in_maps, (TOT_A, TOT_B, offA, offB)


# ----------------------------------------------------------------------------
# Device program
# ----------------------------------------------------------------------------

def build_program(cfg, TOT_A, TOT_B, offA, offB, reps=1, no_coll=False,
                  dbg=False, hoist_y=False, skip_gather=False,
                  gather_only=False, one_queue=False, no_fence=True):
    NW, NT, NWS, PAD_N, GP = cfg.NW, cfg.NT, cfg.NWS, cfg.PAD_N, cfg.GP
    NT_A, NT_B, HALF, CWL = cfg.NT_A, cfg.NT_B, cfg.HALF, cfg.CW
    NCOL = PAD_N // 128          # table "windows" (row groups of 128)
    FLUSH_GI = CWL // cfg.GSZ - 1   # group index after which chunk 0 is done
    rg = [list(range(cfg.C))]

    nc = bacc.Bacc("TRN2", target_bir_lowering=False, debug=False,
                   num_devices=cfg.C, num_swdge_queues=4)

    din = {}
    def inp(name, shape, dt=F32):
        din[name] = nc.dram_tensor(name, list(shape), dt, kind="ExternalInput")
        return din[name]

    inp("ixa", (128, TOT_A), I16)
    inp("ixb", (128, TOT_B), I16)
    inp("ei_dstrel", (128, NW * NT), BF16)
    inp("xt_full", (128, NCOL))
    inp("dinv_full", (128, NCOL))
    inp("x_row", (1, NWS), BF16)
    inp("dinv_row", (1, NWS), BF16)
    inp("dinvT", (64, NWS), BF16)
    inp("nd_batch", (128, NW), BF16)
    inp("iota", (128, GP), BF16)
    inp("cnts", (128, 2))
    inp("w1b", (1, 64), BF16)
    inp("w2b", (64, 128), BF16); inp("w3b", (128, 64), BF16)
    inp("b1", (64, 1)); inp("b2", (128, 1)); inp("b3", (64, 1))
    inp("l1w", (64, 32)); inp("l1b", (32, 1)); inp("l2w", (32, 1))
    inp("l2b", (128, 1))

    out_d = nc.dram_tensor("out", [cfg.GP, 1], F32, kind="ExternalOutput")

    y_full = nc.dram_tensor("y_full", [PAD_N, 128], BF16, kind="Internal")
    g2_sl = nc.dram_tensor("g2_slice", [NWS, 128], BF16, kind="Internal")
    g2_full = nc.dram_tensor("g2_full", [PAD_N, 128], BF16, kind="Internal",
                             addr_space="Shared")
    g3_sl = nc.dram_tensor("g3_slice", [NWS, 128], BF16, kind="Internal")
    g3_full = nc.dram_tensor("g3_full", [PAD_N, 128], BF16, kind="Internal",
                             addr_space="Shared")
    if dbg:
        dbg_selfg2 = nc.dram_tensor("dbg_selfg2", [64, NWS], BF16,
                                    kind="ExternalOutput")
        dbg_g2full = nc.dram_tensor("dbg_g2full", [PAD_N, 128], BF16,
                                    kind="ExternalOutput")
        dbg_selfg3 = nc.dram_tensor("dbg_selfg3", [64, NWS], BF16,
                                    kind="ExternalOutput")
        dbg_pool = nc.dram_tensor("dbg_pool", [GP, 64], F32,
                                  kind="ExternalOutput")
        dbg_poolred = nc.dram_tensor("dbg_poolred", [GP, 64], F32,
                                     kind="ExternalOutput")
    pool_in = nc.dram_tensor("pool_in", [GP, 64], F32, kind="Internal")
    pool_out = nc.dram_tensor("pool_out", [GP, 64], F32, kind="Internal",
                              addr_space="Shared")

    with tile.TileContext(nc) as tc, ExitStack() as ctx:
        P = ctx.enter_context
        setup = P(tc.tile_pool(name="setup", bufs=1))
        oh_pool = P(tc.tile_pool(name="oh", bufs=3))
        gb_pool = P(tc.tile_pool(name="gb", bufs=3))
        fn_pool = P(tc.tile_pool(name="fn", bufs=2))
        yb_pool = P(tc.tile_pool(name="yb", bufs=2))
        psS = P(tc.tile_pool(name="psS", bufs=2, space="PSUM"))
        psZ = P(tc.tile_pool(name="psZ", bufs=2, space="PSUM"))
        psW = P(tc.tile_pool(name="psW", bufs=1, space="PSUM"))
        psT = P(tc.tile_pool(name="psT", bufs=1, space="PSUM"))
        psHold = P(tc.tile_pool(name="psHold", bufs=1, space="PSUM"))
        ev1 = P(tc.tile_pool(name="ev1", bufs=3))
        ev2 = P(tc.tile_pool(name="ev2", bufs=3))
        ev3 = P(tc.tile_pool(name="ev3", bufs=3))
        stg = P(tc.tile_pool(name="stg", bufs=1))

        def load(name, shape, dt=F32):
            t = setup.tile(list(shape), dt, tag=name)
            nc.sync.dma_start(out=t[:], in_=din[name].ap()[:])
            return t

        ixa = load("ixa", (128, TOT_A), I16)
        ixb = load("ixb", (128, TOT_B), I16)
        dstrel = load("ei_dstrel", (128, NW * NT), BF16)
        xt_full = load("xt_full", (128, NCOL))
        dinv_full = load("dinv_full", (128, NCOL))
        x_row = load("x_row", (1, NWS), BF16)
        dinv_row = load("dinv_row", (1, NWS), BF16)
        dinvT = load("dinvT", (64, NWS), BF16)
        nd_batch = load("nd_batch", (128, NW), BF16)
        iota = load("iota", (128, GP), BF16)
        cnts = load("cnts", (128, 2))
        w1b = load("w1b", (1, 64), BF16)
        w2b = load("w2b", (64, 128), BF16)
        w3b = load("w3b", (128, 64), BF16)
        b1 = load("b1", (64, 1)); b2 = load("b2", (128, 1))
        b3 = load("b3", (64, 1))
        l1w = load("l1w", (64, 32)); l1b = load("l1b", (32, 1))
        l2w = load("l2w", (32, 1)); l2b = load("l2b", (128, 1))

        ident = setup.tile([128, 128], F32, tag="ident")
        make_identity(nc, ident[:])
        identb = setup.tile([128, 128], BF16, tag="identb")
        nc.scalar.activation(out=identb[:], in_=ident[:], func=AF.Copy)
        fence_ix = setup.tile([128, 1], I16, tag="fence_ix")
        nc.vector.memset(fence_ix[:], 0)

        # y row vector (for L1 self-term) and full y in permuted-table order
        y_rowB = setup.tile([1, NWS], BF16, tag="y_rowB")
        nc.vector.tensor_tensor(out=y_rowB[:], in0=x_row[:],
                                in1=dinv_row[:], op=OP.mult)
        y_cols = setup.tile([128, NCOL], F32, tag="y_cols")
        nc.vector.tensor_tensor(out=y_cols[:], in0=xt_full[:],
                                in1=dinv_full[:], op=OP.mult)
        y_colsb = setup.tile([128, NCOL], BF16, tag="y_colsb")
        nc.scalar.activation(out=y_colsb[:], in_=y_cols[:], func=AF.Copy)

        staging = stg.tile([128, NW * 128], BF16, tag="staging")
        nc.vector.memset(staging[:], 0)
        selfg2 = stg.tile([64, NWS], BF16, tag="selfg2")
        selfg3 = stg.tile([64, NWS], BF16, tag="selfg3")

        def build_y_table():
            # contiguous full-width rows: strided sub-row DMA writes are
            # several times slower on HW than bulk contiguous writes.
            ychunk = NCOL // 28
            for ci in range(28):
                cs = slice(ci * ychunk, (ci + 1) * ychunk)
                ybuf = yb_pool.tile([128, ychunk * 128], BF16, tag="ybuf")
                nc.vector.tensor_copy(
                    out=ybuf[:].rearrange("p (w f) -> p w f", f=128),
                    in_=y_colsb[:, cs, None].to_broadcast(
                        [128, ychunk, 128]))
                dst = y_full.ap()[ci * ychunk * 128:(ci + 1) * ychunk * 128,
                                  :].rearrange("(w p) f -> p w f", p=128)
                nc.sync.dma_start(out=dst[:],
                                  in_=ybuf[:].rearrange(
                                      "p (w f) -> p w f", f=128))

        def gather_group(gi, gn, table):
            """balanced 4-queue gather: total descs cut in 4 equal runs.

            Returns (gb_tile, fences); fences is a flat list (possibly
            empty) guarding the whole group when fencing is enabled.
            """
            gb = gb_pool.tile([128, cfg.GSZ * NT * 128], BF16, tag="gb")
            nA = gn * NT_A * 128
            nB = gn * NT_B * 128
            total = nA + nB
            per_q = ((total // 4) // 128) * 128
            cuts = [0, per_q, 2 * per_q, 3 * per_q, total]
            fences = []
            for q in range(4):
                s, e = cuts[q], cuts[q + 1]
                qq = 0 if one_queue else q
                calls = []
                a0, a1 = max(s, 0), min(e, nA)
                if a1 > a0:
                    calls.append((True, a0, a1 - a0))
                b0, b1 = max(s, nA) - nA, min(e, total) - nA
                if b1 > b0:
                    calls.append((False, b0, b1 - b0))
                made = []
                for half_lo, s0, n in calls:
                    col0 = s0 if half_lo else nA + s0
                    off_arr = offA if half_lo else offB
                    ic0 = off_arr[gi] + s0 // 16
                    tab = (table.ap()[:HALF, :] if half_lo
                           else table.ap()[HALF:, :])
                    ixt = ixa if half_lo else ixb
                    call = nc.gpsimd.dma_gather(
                        out_ap=gb[:, col0:col0 + n].rearrange(
                            "p (t f) -> p t f", f=128),
                        in_ap=tab,
                        idxs_ap=ixt[:, ic0:ic0 + n // 16],
                        num_idxs=n, num_idxs_reg=n, elem_size=128,
                        single_packet=False, queue_num=qq)
                    made.append(call)
                if not no_fence and made:
                    fence_t = fn_pool.tile([128, 128], BF16, tag=f"fence{q}")
                    fence = nc.gpsimd.dma_gather(
                        out_ap=fence_t[:].rearrange("p (t f) -> p t f", f=128),
                        in_ap=table.ap()[:HALF, :],
                        idxs_ap=fence_ix[:],
                        num_idxs=16, num_idxs_reg=16, elem_size=128,
                        single_packet=True, queue_num=qq)
                    for call in made:
                        add_dep_helper(fence.ins, call.ins, True, "fence>g")
                    fences.append(fence)
            if skip_gather:
                nc.vector.memset(gb[:, 0:16], 0)
            return gb, fences

        def onehot_win(w):
            oh = oh_pool.tile([128, NT * 128], BF16, tag="oh")
            dr3 = dstrel[:, w * NT:(w + 1) * NT][:, :, None].to_broadcast(
                [128, NT, 128])
            io3 = iota[:, None, :128].to_broadcast([128, NT, 128])
            nc.vector.tensor_tensor(
                out=oh[:].rearrange("p (t j) -> p t j", j=128),
                in0=dr3, in1=io3, op=OP.is_equal)
            return oh

        def scatter_group(gb, fences, ws, gn, F):
            """accumulate all gn windows' scatters into one [F, gn*128] psum."""
            ps = psS.tile([F, gn * 128], F32, space="PSUM", tag="psS")
            for wl in range(gn):
                oh = onehot_win(ws + wl)
                for t in range(NT):
                    c = chunk_of(cfg, wl, t, gn)
                    mm = nc.tensor.matmul(
                        out=ps[:, wl * 128:(wl + 1) * 128],
                        lhsT=gb[:, c * 128:c * 128 + F],
                        rhs=oh[:, t * 128:(t + 1) * 128],
                        start=(t == 0), stop=(t == NT - 1))
                    for fence in fences:
                        add_dep_helper(mm.ins, fence.ins, True, "mm>fence")
            return ps

        def gsl(ws, gn):
            return slice(ws * 128, (ws + gn) * 128)

        def stage_flush(gi, sl_t, full_t):
            """after group gi, write finished staging chunk; AllGather it."""
            if gi == FLUSH_GI:
                rsl = slice(0, CWL * 128)
                fsl = slice(0, cfg.CH0)
            elif gi == len(cfg.groups) - 1:
                rsl = slice(CWL * 128, NWS)
                fsl = slice(cfg.CH0, PAD_N)
            else:
                return
            nc.sync.dma_start(
                out=sl_t.ap()[rsl, :].rearrange("(w p) f -> p w f", p=128),
                in_=staging[:, rsl.start:rsl.stop].rearrange(
                    "p (w f) -> p w f", f=128))
            if no_coll:
                nrows = (rsl.stop - rsl.start)
                nc.gpsimd.dma_start(
                    out=full_t.ap()[fsl.start:fsl.start + nrows, :],
                    in_=sl_t.ap()[rsl, :])
            else:
                nc.gpsimd.collective_compute(
                    "AllGather", OP.bypass, replica_groups=rg,
                    ins=[sl_t.ap()[rsl, :]], outs=[full_t.ap()[fsl, :]])

        def win_to_staging(w, src_fm):
            """transpose a [64,128] bf16 feature-major tile into staging."""
            psN = psT.tile([128, 64], BF16, space="PSUM", tag="psN")
            nc.tensor.transpose(out=psN[:], in_=src_fm,
                                identity=identb[:64, :64])
            nc.scalar.activation(out=staging[:, w * 128:w * 128 + 64],
                                 in_=psN[:], func=AF.Copy)

        if hoist_y:
            build_y_table()
        for _rep in range(reps):
            if gather_only:
                for tbl in (y_full, g2_full, g3_full):
                    for gi, (ws, gn) in enumerate(cfg.groups):
                        gather_group(gi, gn, tbl)
                continue
            if not hoist_y:
                build_y_table()

            # ---- Layer 1 ----------------------------------------------------
            for gi, (ws, gn) in enumerate(cfg.groups):
                gb, fences = gather_group(gi, gn, y_full)
                ps1 = scatter_group(gb, fences, ws, gn, 1)
                s1c = ev1.tile([1, gn * 128], BF16, tag="s1c")
                nc.scalar.activation(out=s1c[:], in_=ps1[:], func=AF.Copy)
                s1b = ev1.tile([1, gn * 128], BF16, tag="s1b")
                nc.vector.tensor_tensor(out=s1b[:], in0=s1c[:],
                                        in1=y_rowB[:, gsl(ws, gn)], op=OP.add)
                psO = psZ.tile([64, gn * 128], F32, space="PSUM", tag="psz")
                for wl in range(gn):
                    nc.tensor.matmul(out=psO[:, wl * 128:(wl + 1) * 128],
                                     lhsT=w1b[:],
                                     rhs=s1b[:, wl * 128:(wl + 1) * 128],
                                     start=True, stop=True)
                p1b = ev2.tile([64, gn * 128], BF16, tag="p1b")
                nc.scalar.activation(out=p1b[:], in_=psO[:], func=AF.Copy)
                m1 = ev2.tile([64, gn * 128], BF16, tag="m1")
                nc.vector.tensor_tensor(out=m1[:], in0=p1b[:],
                                        in1=dinvT[:, gsl(ws, gn)], op=OP.mult)
                h1 = ev3.tile([64, gn * 128], BF16, tag="h1")
                nc.scalar.activation(out=h1[:], in_=m1[:], func=AF.Relu,
                                     bias=b1[:])
                nc.vector.tensor_tensor(out=selfg2[:, gsl(ws, gn)], in0=h1[:],
                                        in1=dinvT[:, gsl(ws, gn)], op=OP.mult)
                for wl in range(gn):
                    w = ws + wl
                    win_to_staging(w, selfg2[:, w * 128:(w + 1) * 128])
                stage_flush(gi, g2_sl, g2_full)

            if dbg and _rep == 0:
                nc.sync.dma_start(out=dbg_selfg2.ap()[:], in_=selfg2[:])

            # ---- Layer 2 (+ fold W3, produce g3) ----------------------------
            for gi, (ws, gn) in enumerate(cfg.groups):
                gb, fences = gather_group(gi, gn, g2_full)
                ps2 = scatter_group(gb, fences, ws, gn, 64)
                p2b = ev1.tile([64, gn * 128], BF16, tag="p2b")
                nc.scalar.activation(out=p2b[:], in_=ps2[:], func=AF.Copy)
                tmpa = ev1.tile([64, gn * 128], BF16, tag="tmpa")
                nc.vector.tensor_tensor(out=tmpa[:], in0=p2b[:],
                                        in1=selfg2[:, gsl(ws, gn)], op=OP.add)
                aggT = ev2.tile([64, gn * 128], BF16, tag="aggT")
                nc.vector.tensor_tensor(out=aggT[:], in0=tmpa[:],
                                        in1=dinvT[:, gsl(ws, gn)], op=OP.mult)
                psz = psZ.tile([128, gn * 128], F32, space="PSUM", tag="psz")
                for wl in range(gn):
                    nc.tensor.matmul(out=psz[:, wl * 128:(wl + 1) * 128],
                                     lhsT=w2b[:],
                                     rhs=aggT[:, wl * 128:(wl + 1) * 128],
                                     start=True, stop=True)
                h2 = ev3.tile([128, gn * 128], BF16, tag="h2")
                nc.scalar.activation(out=h2[:], in_=psz[:], func=AF.Relu,
                                     bias=b2[:])
                pst3 = psW.tile([64, gn * 128], F32, space="PSUM", tag="psw")
                for wl in range(gn):
                    nc.tensor.matmul(out=pst3[:, wl * 128:(wl + 1) * 128],
                                     lhsT=w3b[:],
                                     rhs=h2[:, wl * 128:(wl + 1) * 128],
                                     start=True, stop=True)
                t3b = ev1.tile([64, gn * 128], BF16, tag="t3b")
                nc.scalar.activation(out=t3b[:], in_=pst3[:], func=AF.Copy)
                nc.vector.tensor_tensor(out=selfg3[:, gsl(ws, gn)],
                                        in0=t3b[:],
                                        in1=dinvT[:, gsl(ws, gn)], op=OP.mult)
                for wl in range(gn):
                    w = ws + wl
                    win_to_staging(w, selfg3[:, w * 128:(w + 1) * 128])
                stage_flush(gi, g3_sl, g3_full)

            if dbg and _rep == 0:
                nc.sync.dma_start(out=dbg_selfg3.ap()[:], in_=selfg3[:])
                dbt = setup.tile([128, 512], BF16, tag="dbt")
                for blk in range(PAD_N // 512):
                    nc.sync.dma_start(
                        out=dbt[:].rearrange("p (a f) -> p a f", f=128),
                        in_=g2_full.ap()[blk * 512:(blk + 1) * 512, :]
                        .rearrange("(a p) f -> p a f", p=128))
                    nc.sync.dma_start(
                        out=dbg_g2full.ap()[blk * 512:(blk + 1) * 512, :]
                        .rearrange("(a p) f -> p a f", p=128),
                        in_=dbt[:].rearrange("p (a f) -> p a f", f=128))

            # ---- Layer 3 + pooling ------------------------------------------
            pooled_at = psHold.tile([128, 64], F32, space="PSUM",
                                    tag="pool_a")
            pooled_bt = psHold.tile([128, 64], F32, space="PSUM",
                                    tag="pool_b")
            pooled_a = pooled_at[:]
            pooled_b = pooled_bt[:]
            for gi, (ws, gn) in enumerate(cfg.groups):
                gb, fences = gather_group(gi, gn, g3_full)
                ps3 = scatter_group(gb, fences, ws, gn, 64)
                p3b = ev1.tile([64, gn * 128], BF16, tag="p2b")
                nc.scalar.activation(out=p3b[:], in_=ps3[:], func=AF.Copy)
                tmpa = ev1.tile([64, gn * 128], BF16, tag="tmpa")
                nc.vector.tensor_tensor(out=tmpa[:], in0=p3b[:],
                                        in1=selfg3[:, gsl(ws, gn)], op=OP.add)
                agg3 = ev2.tile([64, gn * 128], BF16, tag="aggT")
                nc.vector.tensor_tensor(out=agg3[:], in0=tmpa[:],
                                        in1=dinvT[:, gsl(ws, gn)], op=OP.mult)
                h3 = ev3.tile([64, gn * 128], BF16, tag="h3")
                nc.scalar.activation(out=h3[:], in_=agg3[:], func=AF.Relu,
                                     bias=b3[:])
                for wl in range(gn):
                    w = ws + wl
                    psN = psT.tile([128, 64], BF16, space="PSUM", tag="psN")
                    nc.tensor.transpose(
                        out=psN[:], in_=h3[:, wl * 128:(wl + 1) * 128],
                        identity=identb[:64, :64])
                    h3nm = ev3.tile([128, 64], BF16, tag="h3nm")
                    nc.scalar.activation(out=h3nm[:], in_=psN[:],
                                         func=AF.Copy)
                    ohp = oh_pool.tile([128, GP], BF16, tag="ohp")
                    bc = nd_batch[:, w:w + 1].to_broadcast([128, GP])
                    nc.vector.tensor_tensor(out=ohp[:], in0=bc, in1=iota[:],
                                            op=OP.is_equal)
                    for half, ps_pool in ((0, pooled_a), (1, pooled_b)):
                        lhs = ohp[:, half * 128:(half + 1) * 128]
                        nc.tensor.matmul(out=ps_pool, lhsT=lhs,
                                         rhs=h3nm[:],
                                         start=(w == 0), stop=(w == NW - 1))

            # ---- pool AllReduce ---------------------------------------------
            pa = setup.tile([128, 64], F32, tag="pa")
            pb = setup.tile([128, 64], F32, tag="pb")
            nc.scalar.activation(out=pa[:], in_=pooled_a, func=AF.Copy)
            nc.scalar.activation(out=pb[:], in_=pooled_b, func=AF.Copy)
            nc.sync.dma_start(out=pool_in.ap()[0:128, :], in_=pa[:])
            nc.sync.dma_start(out=pool_in.ap()[128:256, :], in_=pb[:])
            if dbg and _rep == 0:
                nc.sync.dma_start(out=dbg_pool.ap()[0:128, :], in_=pa[:])
                nc.sync.dma_start(out=dbg_pool.ap()[128:256, :], in_=pb[:])
            if no_coll:
                nc.gpsimd.dma_start(out=pool_out.ap()[:],
                                    in_=pool_in.ap()[:])
            else:
                nc.gpsimd.collective_compute(
                    "AllReduce", OP.add, replica_groups=rg,
                    ins=[pool_in.ap()[:]], outs=[pool_out.ap()[:]])

        # ---- finale ---------------------------------------------------------
        if dbg:
            dbr = setup.tile([128, 128], F32, tag="dbr")
            nc.sync.dma_start(
                out=dbr[:].rearrange("p (a f) -> p a f", f=64),
                in_=pool_out.ap()[:].rearrange("(a p) f -> p a f", p=128))
            nc.sync.dma_start(
                out=dbg_poolred.ap()[:].rearrange("(a p) f -> p a f", p=128),
                in_=dbr[:].rearrange("p (a f) -> p a f", f=64))
        meanT = setup.tile([64, 256], F32, tag="meanT")
        for half in (0, 1):
            pl = setup.tile([128, 64], F32, tag=f"pl{half}")
            nc.sync.dma_start(
                out=pl[:], in_=pool_out.ap()[half * 128:(half + 1) * 128, :])
            cntm = setup.tile([128, 1], F32, tag=f"cntm{half}")
            nc.vector.tensor_scalar_max(out=cntm[:],
                                        in0=cnts[:, half:half + 1],
                                        scalar1=1.0)
            rc = setup.tile([128, 1], F32, tag=f"rc{half}")
            nc.vector.reciprocal(out=rc[:], in_=cntm[:])
            mean = setup.tile([128, 64], F32, tag=f"mean{half}")
            nc.vector.tensor_scalar_mul(out=mean[:], in0=pl[:],
                                        scalar1=rc[:])
            psMT = psT.tile([64, 128], F32, space="PSUM", tag="psN")
            nc.tensor.transpose(out=psMT[:], in_=mean[:], identity=ident[:])
            nc.scalar.activation(out=meanT[:, half * 128:(half + 1) * 128],
                                 in_=psMT[:], func=AF.Copy)

        psZ1 = psZ.tile([32, 256], F32, space="PSUM", tag="psz")
        nc.tensor.matmul(out=psZ1[:], lhsT=l1w[:], rhs=meanT[:],
                         start=True, stop=True)
        z1 = setup.tile([32, 256], F32, tag="z1")
        nc.scalar.activation(out=z1[:], in_=psZ1[:], func=AF.Relu,
                             bias=l1b[:])
        for half in (0, 1):
            psO = psT.tile([128, 1], F32, space="PSUM", tag="psN")
            nc.tensor.matmul(out=psO[:],
                             lhsT=z1[:, half * 128:(half + 1) * 128],
                             rhs=l2w[:], start=True, stop=True)
            ob = setup.tile([128, 1], F32, tag=f"ob{half}")
            nc.scalar.activation(out=ob[:], in_=psO[:], func=AF.Identity,
                                 bias=l2b[:])
            nc.sync.dma_start(out=out_d.ap()[half * 128:(half + 1) * 128, :],
                              in_=ob[:])

    nc.compile()
    return nc


# ----------------------------------------------------------------------------
# Runner
# ----------------------------------------------------------------------------

_CACHE = {}


def get_program(cfg, meta, reps=1, no_coll=False, dbg=False, hoist_y=False,
                skip_gather=False, gather_only=False, one_queue=False,
                no_fence=True):
    TOT_A, TOT_B, offA, offB = meta
    key = cfg.key() + (reps, no_coll, dbg, hoist_y, skip_gather, gather_only,
                       one_queue, no_fence)
    if key not in _CACHE:
        _CACHE[key] = build_program(cfg, TOT_A, TOT_B, offA, offB, reps,
                                    no_coll, dbg, hoist_y, skip_gather,
                                    gather_only, one_queue, no_fence)
    return _CACHE[key]


def run(cfg, inputs, trace=False):
    in_maps, meta = prep_inputs(cfg, **inputs)
    nc = get_program(cfg, meta)
    res = bass_utils.run_bass_kernel_spmd(
        nc, in_maps, core_ids=list(range(cfg.C)), trace=trace)
    out = res.results[0]["out"][:cfg.G, :].astype(np.float32)
    return out, res


def make_cfg(inputs, n_nodes=50000, n_edges=600000, n_graphs=256,
             nw=49, half=32768, gsz=GSZ, cw=CW):
    proto = Cfg(n_nodes, n_edges, n_graphs, 8, nw, 1, 1, half, gsz, cw)
    nt_a, nt_b = compute_caps(n_nodes, inputs["edge_index"], proto)
    return Cfg(n_nodes, n_edges, n_graphs, 8, nw, nt_a, nt_b, half, gsz, cw)


def kernel(**inputs) -> np.ndarray:
    cfg = make_cfg(inputs)
    out, _ = run(cfg, inputs)
    return out
